# revision 1
# baseline (speedup 1.0000x reference)
"""Blocked-FP8 linear (dequant + matmul + bias) on 8 Trainium2 NeuronCores.

Computation: out[b,s,o] = sum_i x[b,s,i] * (weight[o,i] * scale_inv[o//128, i//128]) + bias[o]
Shapes: x [2, 2048, 4096] f32, weight [4096, 4096] f32 (e4m3-quantized values),
        weight_scale_inv [32, 32] f32, bias [4096] f32 -> out [2, 2048, 4096] f32.

Sharding: 2-way over tokens x 4-way over out_features (colwise tensor-parallel,
no collectives). Each core computes a [2048 token, 1024 out] block as
out.T = W_deq @ X.T with K(=in_features) on the partition dim.

Host-side work is layout/sharding only: slicing, transposition to K-major,
bf16 wire format (exact for the e4m3-valued weight), and replicating the
per-block scales / bias into per-partition columns. All arithmetic (dequant,
matmul, bias) runs on device.
"""

import os
import sys

for _p in ("/opt/trn_rl_repo", "/root/.axon_site/_ro/trn_rl_repo"):
    if os.path.isdir(_p) and _p not in sys.path:
        sys.path.insert(0, _p)

import ml_dtypes
import numpy as np

import concourse.bass as bass  # noqa: F401  (registers AP machinery)
import concourse.tile as tile
from concourse import bacc, mybir
from concourse.bass_utils import run_bass_kernel_spmd

BLOCK = 128
B, S, IN, OUT = 2, 2048, 4096, 4096
N_CORES = 8
TB_SPLIT = 2            # token split
OB_SPLIT = 4            # out_features split
T_SH = B * S // TB_SPLIT    # 2048 tokens per core
O_SH = OUT // OB_SPLIT      # 1024 out features per core
KB = IN // BLOCK            # 32 k-blocks
JB = O_SH // BLOCK          # 8 local o-blocks
TT = 512                    # matmul moving free dim (tokens per psum tile)
NT = T_SH // TT             # 4 token tiles

_BF16 = ml_dtypes.bfloat16

_compiled = None


def _build_program():
    nc = bacc.Bacc("TRN2", target_bir_lowering=False, debug=False,
                   num_devices=N_CORES)

    xt = nc.dram_tensor("xt", [IN, T_SH], mybir.dt.bfloat16,
                        kind="ExternalInput")
    wt = nc.dram_tensor("wt", [JB, BLOCK, KB, BLOCK], mybir.dt.bfloat16,
                        kind="ExternalInput")
    sc = nc.dram_tensor("sc", [BLOCK, JB * KB], mybir.dt.float32,
                        kind="ExternalInput")
    bc = nc.dram_tensor("bc", [BLOCK, JB], mybir.dt.float32,
                        kind="ExternalInput")
    out = nc.dram_tensor("out", [O_SH, T_SH], mybir.dt.float32,
                         kind="ExternalOutput")

    x_ap = xt.ap().rearrange("(k p) t -> p k t", p=BLOCK)
    out_ap = out.ap()

    with tile.TileContext(nc) as tc:
        with (
            tc.tile_pool(name="consts", bufs=1) as consts,
            tc.tile_pool(name="wpool", bufs=JB) as wpool,
            tc.tile_pool(name="xpool", bufs=3) as xpool,
            tc.tile_pool(name="opool", bufs=4) as opool,
            tc.tile_pool(name="pspool", bufs=4, space="PSUM") as pspool,
        ):
            sc_t = consts.tile([BLOCK, JB * KB], mybir.dt.float32)
            nc.sync.dma_start(out=sc_t[:], in_=sc.ap())
            bc_t = consts.tile([BLOCK, JB], mybir.dt.float32)
            nc.sync.dma_start(out=bc_t[:], in_=bc.ap())

            # Load + dequantize the weight, one o-block slice at a time so the
            # first matmuls only wait on 1 MiB of weight traffic.
            w_tiles = []
            for j in range(JB):
                w_t = wpool.tile([BLOCK, KB, BLOCK], mybir.dt.bfloat16)
                nc.sync.dma_start(out=w_t[:], in_=wt.ap()[j])
                for k in range(KB):
                    nc.vector.tensor_scalar_mul(
                        w_t[:, k, :], w_t[:, k, :],
                        sc_t[:, j * KB + k: j * KB + k + 1])
                w_tiles.append(w_t)

            for ti in range(NT):
                x_t = xpool.tile([BLOCK, KB, TT], mybir.dt.bfloat16)
                nc.sync.dma_start(out=x_t[:],
                                  in_=x_ap[:, :, ti * TT:(ti + 1) * TT])
                for j in range(JB):
                    ps = pspool.tile([BLOCK, TT], mybir.dt.float32)
                    for k in range(KB):
                        nc.tensor.matmul(ps[:], w_tiles[j][:, k, :],
                                         x_t[:, k, :],
                                         start=(k == 0), stop=(k == KB - 1))
                    o_t = opool.tile([BLOCK, TT], mybir.dt.float32)
                    nc.vector.tensor_scalar_add(o_t[:], ps[:],
                                                bc_t[:, j:j + 1])
                    nc.sync.dma_start(
                        out=out_ap[j * BLOCK:(j + 1) * BLOCK,
                                   ti * TT:(ti + 1) * TT],
                        in_=o_t[:])

    nc.compile()
    return nc


def _get_program():
    global _compiled
    if _compiled is None:
        _compiled = _build_program()
    return _compiled


def _shard_inputs(x, weight, weight_scale_inv, bias):
    x_flat = np.ascontiguousarray(x.reshape(B * S, IN))
    in_maps = []
    for c in range(N_CORES):
        tb, ob = divmod(c, OB_SPLIT)
        x_sh = x_flat[tb * T_SH:(tb + 1) * T_SH, :]          # [T_SH, IN]
        xt = np.ascontiguousarray(x_sh.T).astype(_BF16)      # [IN, T_SH]

        w_sh = weight[ob * O_SH:(ob + 1) * O_SH, :]          # [O_SH, IN]
        # wt[j, p, k, o] = w_sh[j*128 + o, k*128 + p]
        wt = np.ascontiguousarray(
            w_sh.reshape(JB, BLOCK, KB, BLOCK).transpose(0, 3, 2, 1)
        ).astype(_BF16)

        s_sh = weight_scale_inv[ob * JB:(ob + 1) * JB, :]    # [JB, KB]
        sc = np.ascontiguousarray(
            np.broadcast_to(s_sh.reshape(1, JB * KB), (BLOCK, JB * KB))
        ).astype(np.float32)

        b_sh = bias[ob * O_SH:(ob + 1) * O_SH]               # [O_SH]
        bc = np.ascontiguousarray(
            b_sh.reshape(JB, BLOCK).T).astype(np.float32)    # [128, JB]

        in_maps.append({"xt": xt, "wt": wt, "sc": sc, "bc": bc})
    return in_maps


def _run(in_maps, trace=False):
    nc = _get_program()
    return run_bass_kernel_spmd(nc, in_maps, list(range(N_CORES)),
                                trace=trace)


def _assemble(results):
    out_full = np.empty((B * S, OUT), dtype=np.float32)
    for c in range(N_CORES):
        tb, ob = divmod(c, OB_SPLIT)
        out_c = np.asarray(results[c]["out"], dtype=np.float32)  # [O_SH, T_SH]
        out_full[tb * T_SH:(tb + 1) * T_SH,
                 ob * O_SH:(ob + 1) * O_SH] = out_c.T
    return out_full.reshape(B, S, OUT)


def kernel(x, weight, weight_scale_inv, bias):
    x = np.asarray(x, dtype=np.float32)
    weight = np.asarray(weight, dtype=np.float32)
    weight_scale_inv = np.asarray(weight_scale_inv, dtype=np.float32)
    bias = np.asarray(bias, dtype=np.float32)
    assert x.shape == (B, S, IN), x.shape
    assert weight.shape == (OUT, IN), weight.shape
    assert weight_scale_inv.shape == (OUT // BLOCK, IN // BLOCK)
    assert bias.shape == (OUT,)

    in_maps = _shard_inputs(x, weight, weight_scale_inv, bias)
    res = _run(in_maps)
    return _assemble(res.results)
